# revision 28
# baseline (speedup 1.0000x reference)
"""Multi-head attention kernel for Trainium2, 8-core tensor/data parallel.

Problem: x[2,2048,1024] -> qkv proj (w_qkv [1024,3072]) -> 16-head attention
         -> out proj (w_proj [1024,1024]) + b_proj.

Sharding: core c handles batch b=c//4 and heads 4*(c%4)..4*(c%4)+4.
Each core computes a partial output Y^T = w_proj_rows^T @ OH (its 4 heads'
contribution, transposed); the host sums the 4 partials per batch,
transposes, and adds the bias.

Schedule: single slot stream paced by the softmax-exp chain on the ACT
engine (exp only runs there; ~1.05us per [128,2,512] tile).  Each slot
emits the S^T matmuls for one (it, kt) tile as a 2x2 quadrant-tiled group
(4x [64dx64k] stationaries at tile positions (0/64, 0/64) run concurrently
on the PE array), the exp for that tile, and the PV matmul pair for the
slot LAG positions earlier; Q/K/V projections and the output projection
are woven into the remaining PE capacity between slots using a ns-budget
model so the PE queue never runs ahead of the ACT queue.

All host<->device tensors are pre-arranged on the host so every DMA is a
contiguous [128, *] block (no strided descriptor storms); the kernel
writes the output in raw [qb, ct, p, n] layout and the host unscrambles.

All data is bf16 (PSUM accumulation fp32).  Softmax skips max-subtraction
(scores are ~N(0,1) after the 1/sqrt(D) scale) and folds the row-sum into
the PV matmul via an appended ones-column on V; denominators use the fast
approximate reciprocal.  Normalization multiplies read the PV PSUM
accumulators directly (no staging copies), keeping the ACT engine free
for exp.
"""

from contextlib import ExitStack

import numpy as np

import concourse.bass as bass
import concourse.mybir as mybir
from concourse import bacc, tile

B, N, C, H = 2, 2048, 1024, 16
D = C // H            # 64 head dim
SCALE = float(D) ** -0.5
HPC = 4               # heads per core
HD = HPC * D          # 256 head-dim columns per core
NCORES = 8

F32 = mybir.dt.float32
BF16 = mybir.dt.bfloat16

QT = N // 128         # 16 query/key 128-tiles
CT = C // 128         # 8 channel 128-tiles
QB = N // 512         # 4 query 512-blocks
HDT = HD // 128       # 2 head-pair tiles (2 heads of 64 each)

NIT = HDT * QB        # 8 attention iterations (ht-major)
LAG = 12              # PV trails its exp by this many slots
LAST_LAG = 2          # tighter drain for the final iteration

# empirical effective PE costs (ns), from HW trace at nominal clock
SPAIR_NS = 320.0      # row-tiled S matmul pair (concurrent halves)
PV_NS = 225.0         # one PV matmul, ap=512
PROJ_MM_NS = 240.0    # projection matmul, ap=512
EXP_NS = 1060.0       # ACT ns per [128,2,512] exp
AHEAD_NS = 2000.0     # how far PE emission may run ahead of ACT


def _build():
    nc = bacc.Bacc(None)
    # host-prearranged contiguous layouts (see kernel() for the math)
    xq_d = nc.declare_dram_parameter("xq", [QB, 128, CT, 512], BF16,
                                     isOutput=False)
    # first-token-half of x qb0, its own param so the critical first DMA
    # is small and contiguous
    xq0a_d = nc.declare_dram_parameter("xq0a", [128, CT, 256], BF16,
                                       isOutput=False)
    xq0b_d = nc.declare_dram_parameter("xq0b", [128, CT, 256], BF16,
                                       isOutput=False)
    # wk/wq split by head-pair tile (ht-major) so ht0 lands first
    wk_d = nc.declare_dram_parameter("wk", [HDT, 128, CT, 128], BF16,
                                     isOutput=False)
    wq_d = nc.declare_dram_parameter("wq", [HDT, 128, CT, 128], BF16,
                                     isOutput=False)
    wv_d = nc.declare_dram_parameter("wv", [128, CT, HD], BF16,
                                     isOutput=False)
    wp_d = nc.declare_dram_parameter("wp", [128, HDT, C], BF16,
                                     isOutput=False)
    y_d = nc.declare_dram_parameter("y", [QB, CT, 128, 512], BF16,
                                    isOutput=True)

    with tile.TileContext(nc) as tc, ExitStack() as ctx:
        const_pool = ctx.enter_context(tc.tile_pool(name="const", bufs=1))
        w_pool = ctx.enter_context(tc.tile_pool(name="w", bufs=1))
        x_pool = ctx.enter_context(tc.tile_pool(name="x", bufs=1))
        qk_pool = ctx.enter_context(tc.tile_pool(name="qk", bufs=1))
        vo_pool = ctx.enter_context(tc.tile_pool(name="vo", bufs=1))
        oht_pool = ctx.enter_context(tc.tile_pool(name="oht", bufs=1))
        pt_pool = ctx.enter_context(tc.tile_pool(name="pt", bufs=1))
        small_pool = ctx.enter_context(tc.tile_pool(name="small", bufs=2))
        out_pool = ctx.enter_context(tc.tile_pool(name="out", bufs=3))
        st_pool = ctx.enter_context(
            tc.tile_pool(name="ps_st", bufs=2, space="PSUM"))
        ot_pool = ctx.enter_context(
            tc.tile_pool(name="ps_ot", bufs=1, space="PSUM"))
        proj_pool = ctx.enter_context(
            tc.tile_pool(name="ps_proj", bufs=2, space="PSUM"))

        ones_f = const_pool.tile([128, 64], F32)
        nc.vector.memset(ones_f, 1.0)
        ones_b = const_pool.tile([128, 64], BF16)
        nc.vector.memset(ones_b, 1.0)

        # ---- input DMAs (all contiguous), ordered by first use ----
        # x qb0 split by token half (one tile per DMA so nothing waits on
        # the other half); qb1-3 are single whole-block tiles
        xq0_t = [x_pool.tile([128, CT, 256], BF16, name=f"xq0{h}",
                             tag=f"xq0{h}") for h in range(2)]
        xq_t = [None] + [x_pool.tile([128, CT, 512], BF16, name=f"xq{qb}",
                                     tag=f"xq{qb}") for qb in range(1, QB)]
        wtiles = {}

        def xq_parts(qb, ct):
            # [(moving_ap, lo, hi)] token chunks covering qb's 512 tokens
            if qb == 0:
                return [(xq0_t[0][:, ct, :], 0, 256),
                        (xq0_t[1][:, ct, :], 256, 512)]
            return [(xq_t[qb][:, ct, :], 0, 512)]

        def xq_slice(qb, ct, ks):
            # [128, 128] token slice (for the v projection stationary)
            if qb == 0:
                h, off = divmod(ks, 256)
                return xq0_t[h][:, ct, off:off + 128]
            return xq_t[qb][:, ct, ks:ks + 128]

        # wk0 + xq0a gate the first projection: first on their queues
        nc.sync.dma_start(out=xq0_t[0], in_=xq0a_d[:, :, :])
        wk_ts, wq_ts = [], []
        for ht in range(HDT):
            for name, dram, lst in (("wk", wk_d, wk_ts), ("wq", wq_d,
                                                          wq_ts)):
                t = w_pool.tile([128, CT, 128], BF16, name=f"{name}{ht}",
                                tag=f"{name}{ht}")
                nc.scalar.dma_start(out=t, in_=dram[ht])
                lst.append(t)
        wtiles["wk"], wtiles["wq"] = wk_ts, wq_ts
        nc.sync.dma_start(out=xq0_t[1], in_=xq0b_d[:, :, :])
        for qb in (1, 2):
            nc.sync.dma_start(out=xq_t[qb], in_=xq_d[qb])
        wv_t = w_pool.tile([128, CT, HD], BF16, name="wv", tag="wv")
        nc.scalar.dma_start(out=wv_t, in_=wv_d[:, :, :])
        wtiles["wv"] = wv_t
        nc.scalar.dma_start(out=(xq_t[3]), in_=xq_d[3])
        wp_full = w_pool.tile([128, HDT, C], BF16, name="wp", tag="wp")
        nc.sync.dma_start(out=wp_full, in_=wp_d[:, :, :])

        # preload the exp table set while DMAs land
        warm = const_pool.tile([128, 64], BF16)
        nc.scalar.activation(warm, ones_f, mybir.ActivationFunctionType.Exp,
                             scale=0.0)

        # ---- persistent activations (all bf16) ----
        qT_t = [qk_pool.tile([128, N], BF16, name=f"qT{i}", tag=f"qT{i}")
                for i in range(HDT)]
        kT_t = [qk_pool.tile([128, N], BF16, name=f"kT{i}", tag=f"kT{i}")
                for i in range(HDT)]
        vo_t = [vo_pool.tile([128, HPC * (D + 1)], BF16, name=f"vo{i}",
                             tag=f"vo{i}") for i in range(QT)]
        oht_t = [oht_pool.tile([128, N], BF16, name=f"oht{i}", tag=f"oht{i}")
                 for i in range(HDT)]
        pt_t = [pt_pool.tile([128, 2, 512], BF16, name=f"pt{i}",
                             tag=f"pt{i}") for i in range(QT)]

        for t in vo_t:
            ones_col = t.rearrange("p (h e) -> p h e", h=HPC)[:, :, D:D + 1]
            nc.gpsimd.tensor_copy(
                ones_col, ones_b[:, 0:HPC].rearrange("p (h o) -> p h o", o=1))

        # ---- work quanta ----
        # projections are emitted as ~0.5us pieces so they weave between
        # S-pairs without stalling the S->exp chain (the st double-buffer
        # only tolerates ~2 slots of PE detour).  Pieces of one quantum
        # share a PSUM tile via `holder` and run in deadline order (stable
        # sort keeps same-deadline insertion order).  part-major within a
        # group: a start=True matmul clears has_written for the WHOLE bank,
        # so the two token-half accumulation groups must not interleave
        # (completed data survives the bit-clear).
        def q_projqk_pieces(ht, qb, dst_t, wname):
            w_full = wtiles[wname][ht]
            cs = slice(qb * 512, (qb + 1) * 512)
            holder = {}
            nparts = len(xq_parts(qb, 0))
            step = 2 if nparts == 1 else 4      # ~480ns per piece
            jobs = []          # (pi, ct_lo, ct_hi)
            for pi in range(nparts):
                for clo in range(0, CT, step):
                    jobs.append((pi, clo, clo + step))
            pieces = []
            for ji, (pi, clo, chi) in enumerate(jobs):
                def go(ji=ji, pi=pi, clo=clo, chi=chi):
                    if ji == 0:
                        holder["ps"] = proj_pool.tile(
                            [128, 512], F32, name="proj", tag="proj")
                    ps = holder["ps"]
                    for ct in range(clo, chi):
                        part, lo, hi = xq_parts(qb, ct)[pi]
                        nc.tensor.matmul(
                            ps[:, lo:hi],
                            w_full[:, ct, :],
                            part,
                            start=(ct == 0), stop=(ct == CT - 1))
                    if ji == len(jobs) - 1:
                        nc.vector.tensor_copy(dst_t[ht][:, cs], ps)
                cost = (chi - clo) * PROJ_MM_NS * (256 if nparts == 2
                                                   else 512) / 512.0
                pieces.append((go, cost))
            return pieces

        def push_projqk(dl, ht, qb, dst_t, wname):
            for i, piece in enumerate(q_projqk_pieces(ht, qb, dst_t,
                                                      wname)):
                push(dl + i, (wname, ht, qb, i), piece)

        def q_projv_pieces(kt):
            holder = {}
            pieces = []
            for ji, clo in enumerate(range(0, CT, CT // 2)):
                def go(ji=ji, clo=clo):
                    qbk, off = divmod(kt * 128, 512)
                    if ji == 0:
                        holder["ps"] = proj_pool.tile(
                            [128, 512], F32, name="proj", tag="proj")
                    ps = holder["ps"]
                    for ct in range(clo, clo + CT // 2):
                        nc.tensor.matmul(ps[:, 0:HD],
                                         xq_slice(qbk, ct, off),
                                         wtiles["wv"][:, ct, :],
                                         start=(ct == 0),
                                         stop=(ct == CT - 1))
                    if ji == 1:
                        vo_view = vo_t[kt].rearrange("p (h e) -> p h e",
                                                     h=HPC)
                        ps_view = ps[:, 0:HD].rearrange("p (h d) -> p h d",
                                                        h=HPC)
                        nc.vector.tensor_copy(vo_view[:, :, 0:D], ps_view)
                pieces.append((go, (CT // 2) * PROJ_MM_NS / 2.0))
            return pieces

        def q_projout(qb, ct):
            def go():
                qs = slice(qb * 512, (qb + 1) * 512)
                cs = slice(ct * 128, (ct + 1) * 128)
                ps = proj_pool.tile([128, 512], F32, name="proj", tag="proj")
                for ht in range(HDT):
                    nc.tensor.matmul(
                        ps, wp_full[:, ht, cs], oht_t[ht][:, qs],
                        start=(ht == 0), stop=(ht == HDT - 1))
                yo = out_pool.tile([128, 512], BF16, name="yo", tag="yo")
                if qb >= QB - 2 and ct % 2 == 0:
                    # tail blocks run post-exp: alternate the PSUM->SBUF
                    # casts between the idle ACT engine and DVE
                    nc.scalar.copy(yo, ps)
                else:
                    nc.vector.tensor_copy(yo, ps)
                nc.sync.dma_start(out=y_d[qb, ct], in_=yo)
            return go, HDT * PROJ_MM_NS

        # ---- slot-stream emission ----
        state = {"pe": 0.0, "act": 0.0}
        fifo = []          # [(deadline, qid, go, cost), ...] kept sorted
        done_ids = set()

        def push(deadline, qid, quantum):
            go, cost = quantum
            fifo.append([deadline, qid, go, cost])
            fifo.sort(key=lambda e: e[0])

        def run_item(item):
            _, qid, go, cost = item
            go()
            state["pe"] += cost
            done_ids.add(qid)

        def force_until(g):
            while fifo and fifo[0][0] <= g:
                run_item(fifo.pop(0))

        def budget_drain():
            while fifo and state["pe"] + fifo[0][3] <= state["act"] + AHEAD_NS:
                run_item(fifo.pop(0))

        # prologue projections for (ht0, qb0): kT tokens 0-255 first (covers
        # kt0/kt1, needs only the xq0a DMA), then the kT remainder, then
        # full qT -- the first S pair only needs the narrow kT chunk + qT
        def projqk_chunk(dst_t, wname, half):
            lo, hi = half * 256, half * 256 + 256
            w_full = wtiles[wname][0]
            ps = proj_pool.tile([128, 512], F32, name="proj", tag="proj")
            for ct in range(CT):
                nc.tensor.matmul(
                    ps[:, 0:256],
                    w_full[:, ct, :],
                    xq0_t[half][:, ct, :],
                    start=(ct == 0), stop=(ct == CT - 1))
            nc.vector.tensor_copy(dst_t[0][:, lo:hi], ps[:, 0:256])
            state["pe"] += CT * PROJ_MM_NS / 2.0

        projqk_chunk(kT_t, "wk", 0)
        projqk_chunk(kT_t, "wk", 1)
        for go, cost in q_projqk_pieces(0, 0, qT_t, "wq"):
            run_item([0, None, go, cost])

        # weave queue: deadlines in global slot units, a few slots before
        # first use; pieces spread one per slot
        for qbk in range(1, QB):
            push_projqk(max(1, 4 * qbk - 4), 0, qbk, kT_t, "wk")
        for kt in range(QT):
            for i, piece in enumerate(q_projv_pieces(kt)):
                push(kt + LAG - 3 + i, ("v", kt, i), piece)
        for qb in range(1, QB):
            push_projqk(16 * qb - 5, 0, qb, qT_t, "wq")
        for qbk in range(QB):
            push_projqk(42 + 5 * qbk, 1, qbk, kT_t, "wk")
        for qb in range(QB):
            push_projqk(64 + 16 * qb - 7, 1, qb, qT_t, "wq")

        iters = [(ht, qb) for ht in range(HDT) for qb in range(QB)]
        ots_by_it = {}
        norm_pending = []
        normb_pending = []
        pv_sched = {}
        for it in range(NIT):
            for kt in range(QT):
                if it == NIT - 1:
                    # drain fast, but not before it-1's PSUM accumulators
                    # are staged out (norm_a(it-1) runs at slot ~123), and
                    # at most ~2 pairs per slot so the burst never jams the
                    # in-order PE FIFO ahead of the last S pairs
                    g = max(it * QT + LAG + kt // 2, it * QT + kt +
                            LAST_LAG)
                else:
                    # first PVs wait until the previous iteration's staging
                    # copies (norm_a at +11, ~1.4us on DVE) free the ot
                    # PSUM banks
                    g = it * QT + max(kt + LAG, 14)
                pv_sched.setdefault(g, []).append((it, kt))

        def emit_s(it, kt):
            ht, qb = iters[it]
            qs = slice(qb * 512, (qb + 1) * 512)
            st = st_pool.tile([128, 2, 512], F32, name="st", tag="st",
                              bufs=2)
            # row-tiled halves (tile positions (0,0)/(64,0)) run
            # concurrently on the PE array
            for hp in range(2):
                prow = slice(hp * 64, hp * 64 + 64)
                nc.tensor.matmul(
                    st[:, hp, :],
                    kT_t[ht][prow, kt * 128:(kt + 1) * 128],
                    qT_t[ht][prow, qs])
            state["pe"] += SPAIR_NS
            return st

        def emit_exp(kt, st):
            nc.scalar.activation(
                pt_t[kt], st, mybir.ActivationFunctionType.Exp, scale=SCALE)
            state["act"] += EXP_NS

        def emit_pv(it, kt):
            ht, qb = iters[it]
            if kt == 0:
                ots_by_it[it] = [
                    ot_pool.tile([65, 512], F32, name=f"ot{hp}",
                                 tag=f"ot{hp}", bufs=1)
                    for hp in range(2)]
            ots = ots_by_it[it]
            for hp in range(2):
                h = 2 * ht + hp
                nc.tensor.matmul(
                    ots[hp],
                    vo_t[kt][:, h * (D + 1):(h + 1) * (D + 1)],
                    pt_t[kt][:, hp, :],
                    start=(kt == 0), stop=(kt == QT - 1))
            state["pe"] += 2 * PV_NS
            if kt == QT - 1:
                norm_pending.append(it)

        def emit_norm_a(it):
            # free the PSUM accumulators ASAP on DVE (keeping the ACT
            # engine's exp chain gap-free); one [65,512] copy per head
            # grabs O rows + the ones-column rowsum together
            ots = ots_by_it.pop(it)
            stgs = []
            for hp in range(2):
                stg = small_pool.tile([65, 512], F32, name=f"stg{hp}",
                                      tag=f"stg{hp}")
                nc.vector.tensor_copy(stg, ots[hp][0:65, :])
                stgs.append(stg)
            return stgs

        def emit_norm_b(it, stgs):
            ht, qb = iters[it]
            qs = slice(qb * 512, (qb + 1) * 512)
            last = it == NIT - 1
            rbs = []
            for hp in range(2):
                # rowsum to its own partition-0 tile: reciprocal_approx_fast
                # breaks on nonzero base partitions as well as on PSUM reads
                sdb = small_pool.tile([1, 512], F32, name=f"sd{hp}",
                                      tag=f"sd{hp}")
                nc.vector.tensor_copy(sdb, stgs[hp][64:65, :])
                r32 = small_pool.tile([1, 512], F32, name=f"r32{hp}",
                                      tag=f"r32{hp}")
                nc.vector.reciprocal_approx_fast(r32, sdb)
                if last:
                    # tail: the PE is idle and the ot PSUM banks are free
                    # after norm_a -- a ones-matmul replicates 1/rowsum in
                    # ~0.2us (vs ~1us GPSIMD broadcast on the latency
                    # chain to the final output block)
                    rb65 = ot_pool.tile([65, 512], F32, name=f"rbp{hp}",
                                        tag=f"ot{hp}")
                    rb = rb65[0:64, :]
                    nc.tensor.matmul(rb, ones_f[0:1, 0:64], r32)
                else:
                    # replicate 1/rowsum across partitions on the idle
                    # GPSIMD engine; keeps norm off the PE mid-stream
                    rb = small_pool.tile([64, 512], F32, name=f"rb{hp}",
                                         tag=f"rb{hp}")
                    nc.gpsimd.partition_broadcast(rb, r32)
                rbs.append(rb)
            for hp in range(2):
                prow = slice(hp * 64, hp * 64 + 64)
                dst = oht_t[ht][prow, qs]
                with nc.allow_low_precision(reason="bf16 attention out"):
                    nc.vector.tensor_mul(dst, stgs[hp][0:64, :], rbs[hp])
            if ht == HDT - 1:
                base = (4 + qb) * 16 + 32
                for ct in range(CT):
                    dl = base + 2 * ct if qb < QB - 1 else 10 ** 6
                    push(dl, ("out", qb, ct), q_projout(qb, ct))

        # slots are emitted in PAIRS: both S-pairs back-to-back (the second
        # pair's half-row weight loads ride the first pair's streams), then
        # both exps, then the lagged PVs -- this halves the costly
        # all-row -> half-row PE transitions
        total_slots = NIT * QT
        for g in range(0, total_slots + 8, 2):
            force_until(g)
            for g2 in (g, g + 1):
                if g2 < total_slots:
                    it, kt = divmod(g2, QT)
                    # exp emitted right after its S pair (ACT queue) while
                    # the two S pairs stay adjacent in the PE FIFO
                    emit_exp(kt, emit_s(it, kt))
            for g2 in (g, g + 1):
                for gpv in pv_sched.pop(g2, ()):
                    emit_pv(*gpv)
            if norm_pending:
                itn = norm_pending.pop(0)
                normb_pending.append((g + 2, itn, emit_norm_a(itn)))
            if normb_pending and normb_pending[0][0] <= g:
                _, itn, sdbs = normb_pending.pop(0)
                emit_norm_b(itn, sdbs)
            budget_drain()
        for g2 in sorted(pv_sched):
            for gpv in pv_sched[g2]:
                emit_pv(*gpv)
        if norm_pending:
            itn = norm_pending.pop(0)
            emit_norm_b(itn, emit_norm_a(itn))
        while normb_pending:
            _, itn, sdbs = normb_pending.pop(0)
            emit_norm_b(itn, sdbs)
        while fifo:
            run_item(fifo.pop(0))

    nc.finalize()
    return nc


_NC_CACHE = None
TRACE = False
LAST_RESULTS = None


def _get_nc():
    global _NC_CACHE
    if _NC_CACHE is None:
        _NC_CACHE = _build()
    return _NC_CACHE


def kernel(x, w_qkv, w_proj, b_proj):
    global LAST_RESULTS
    import ml_dtypes
    from concourse.bass_utils import run_bass_kernel_spmd

    BF = ml_dtypes.bfloat16
    x = np.asarray(x, dtype=np.float32)
    w_qkv = np.asarray(w_qkv, dtype=np.float32)
    w_proj = np.asarray(w_proj, dtype=np.float32)
    b_proj = np.asarray(b_proj, dtype=np.float32)

    nc = _get_nc()

    # xq[qb, p, ct, n] = x[b, qb*512+n, ct*128+p]
    xq_b = []
    for b in range(B):
        xT = x[b].T.astype(BF)                       # [C, N]
        xq_b.append(np.ascontiguousarray(
            xT.reshape(CT, 128, QB, 512).transpose(2, 1, 0, 3)))

    def pre_w(w):                                    # [C, HD] -> [p, ct, HD]
        return np.ascontiguousarray(
            w.reshape(CT, 128, HD).transpose(1, 0, 2).astype(BF))

    def pre_w_ht(w):                           # [C, HD] -> [ht, p, ct, 128]
        return np.ascontiguousarray(
            w.reshape(CT, 128, HDT, 128).transpose(2, 1, 0, 3).astype(BF))

    in_maps = []
    for c in range(NCORES):
        b, g = divmod(c, NCORES // B)
        hs = slice(g * HD, (g + 1) * HD)
        in_maps.append({
            "xq": xq_b[b],
            "xq0a": np.ascontiguousarray(xq_b[b][0][:, :, 0:256]),
            "xq0b": np.ascontiguousarray(xq_b[b][0][:, :, 256:512]),
            "wk": pre_w_ht(w_qkv[:, 1 * C:2 * C][:, hs]),
            "wq": pre_w_ht(w_qkv[:, 0 * C:1 * C][:, hs]),
            "wv": pre_w(w_qkv[:, 2 * C:3 * C][:, hs]),
            # wp[p, ht, c] = w_proj[g*HD + ht*128 + p, c]
            "wp": np.ascontiguousarray(
                w_proj[g * HD:(g + 1) * HD, :]
                .reshape(HDT, 128, C).transpose(1, 0, 2).astype(BF)),
        })
    res = run_bass_kernel_spmd(nc, in_maps, list(range(NCORES)), trace=TRACE)
    LAST_RESULTS = res
    out = np.empty((B, N, C), dtype=np.float32)
    ncb = NCORES // B
    for b in range(B):
        acc = None
        for g in range(ncb):
            y = res.results[b * ncb + g]["y"].astype(np.float32)
            acc = y if acc is None else acc + y
        # y[qb, ct, p, n] -> yT[ct*128+p, qb*512+n]
        yT = acc.transpose(1, 2, 0, 3).reshape(C, N)
        out[b] = yT.T + b_proj
    return out


# revision 29
# speedup vs baseline: 1.0358x; 1.0358x over previous
"""Multi-head attention kernel for Trainium2, 8-core tensor/data parallel.

Problem: x[2,2048,1024] -> qkv proj (w_qkv [1024,3072]) -> 16-head attention
         -> out proj (w_proj [1024,1024]) + b_proj.

Sharding: core c handles batch b=c//4 and heads 4*(c%4)..4*(c%4)+4.
Each core computes a partial output Y^T = w_proj_rows^T @ OH (its 4 heads'
contribution, transposed); the host sums the 4 partials per batch,
transposes, and adds the bias.

Schedule: single slot stream paced by the softmax-exp chain on the ACT
engine (exp only runs there; ~1.05us per [128,2,512] tile).  Each slot
emits the S^T matmuls for one (it, kt) tile as a 2x2 quadrant-tiled group
(4x [64dx64k] stationaries at tile positions (0/64, 0/64) run concurrently
on the PE array), the exp for that tile, and the PV matmul pair for the
slot LAG positions earlier; Q/K/V projections and the output projection
are woven into the remaining PE capacity between slots using a ns-budget
model so the PE queue never runs ahead of the ACT queue.

All host<->device tensors are pre-arranged on the host so every DMA is a
contiguous [128, *] block (no strided descriptor storms); the kernel
writes the output in raw [qb, ct, p, n] layout and the host unscrambles.

All data is bf16 (PSUM accumulation fp32).  Softmax skips max-subtraction
(scores are ~N(0,1) after the 1/sqrt(D) scale) and folds the row-sum into
the PV matmul via an appended ones-column on V; denominators use the fast
approximate reciprocal.  Normalization multiplies read the PV PSUM
accumulators directly (no staging copies), keeping the ACT engine free
for exp.
"""

from contextlib import ExitStack

import numpy as np

import concourse.bass as bass
import concourse.mybir as mybir
from concourse import bacc, tile

B, N, C, H = 2, 2048, 1024, 16
D = C // H            # 64 head dim
SCALE = float(D) ** -0.5
HPC = 4               # heads per core
HD = HPC * D          # 256 head-dim columns per core
NCORES = 8

F32 = mybir.dt.float32
BF16 = mybir.dt.bfloat16

QT = N // 128         # 16 query/key 128-tiles
CT = C // 128         # 8 channel 128-tiles
QB = N // 512         # 4 query 512-blocks
HDT = HD // 128       # 2 head-pair tiles (2 heads of 64 each)

NIT = HDT * QB        # 8 attention iterations (ht-major)
LAG = 12              # PV trails its exp by this many slots
LAST_LAG = 2          # tighter drain for the final iteration

# empirical effective PE costs (ns), from HW trace at nominal clock
SPAIR_NS = 320.0      # row-tiled S matmul pair (concurrent halves)
PV_NS = 225.0         # one PV matmul, ap=512
PROJ_MM_NS = 240.0    # projection matmul, ap=512
EXP_NS = 1060.0       # ACT ns per [128,2,512] exp
AHEAD_NS = 2400.0     # how far PE emission may run ahead of ACT


def _build():
    nc = bacc.Bacc(None)
    # host-prearranged contiguous layouts (see kernel() for the math)
    xq_d = nc.declare_dram_parameter("xq", [QB, 128, CT, 512], BF16,
                                     isOutput=False)
    # first-token-half of x qb0, its own param so the critical first DMA
    # is small and contiguous
    xq0a_d = nc.declare_dram_parameter("xq0a", [128, CT, 256], BF16,
                                       isOutput=False)
    xq0b_d = nc.declare_dram_parameter("xq0b", [128, CT, 256], BF16,
                                       isOutput=False)
    # wk/wq split by head-pair tile (ht-major) so ht0 lands first
    wk_d = nc.declare_dram_parameter("wk", [HDT, 128, CT, 128], BF16,
                                     isOutput=False)
    wq_d = nc.declare_dram_parameter("wq", [HDT, 128, CT, 128], BF16,
                                     isOutput=False)
    wv_d = nc.declare_dram_parameter("wv", [128, CT, HD], BF16,
                                     isOutput=False)
    wp_d = nc.declare_dram_parameter("wp", [128, HDT, C], BF16,
                                     isOutput=False)
    y_d = nc.declare_dram_parameter("y", [QB, CT, 128, 512], BF16,
                                    isOutput=True)

    with tile.TileContext(nc) as tc, ExitStack() as ctx:
        const_pool = ctx.enter_context(tc.tile_pool(name="const", bufs=1))
        w_pool = ctx.enter_context(tc.tile_pool(name="w", bufs=1))
        x_pool = ctx.enter_context(tc.tile_pool(name="x", bufs=1))
        qk_pool = ctx.enter_context(tc.tile_pool(name="qk", bufs=1))
        vo_pool = ctx.enter_context(tc.tile_pool(name="vo", bufs=1))
        oht_pool = ctx.enter_context(tc.tile_pool(name="oht", bufs=1))
        pt_pool = ctx.enter_context(tc.tile_pool(name="pt", bufs=1))
        small_pool = ctx.enter_context(tc.tile_pool(name="small", bufs=2))
        out_pool = ctx.enter_context(tc.tile_pool(name="out", bufs=6))
        st_pool = ctx.enter_context(
            tc.tile_pool(name="ps_st", bufs=2, space="PSUM"))
        ot_pool = ctx.enter_context(
            tc.tile_pool(name="ps_ot", bufs=1, space="PSUM"))
        proj_pool = ctx.enter_context(
            tc.tile_pool(name="ps_proj", bufs=2, space="PSUM"))

        ones_f = const_pool.tile([128, 64], F32)
        nc.vector.memset(ones_f, 1.0)
        ones_b = const_pool.tile([128, 64], BF16)
        nc.vector.memset(ones_b, 1.0)

        # ---- input DMAs (all contiguous), ordered by first use ----
        # x qb0 split by token half (one tile per DMA so nothing waits on
        # the other half); qb1-3 are single whole-block tiles
        xq0_t = [x_pool.tile([128, CT, 256], BF16, name=f"xq0{h}",
                             tag=f"xq0{h}") for h in range(2)]
        xq_t = [None] + [x_pool.tile([128, CT, 512], BF16, name=f"xq{qb}",
                                     tag=f"xq{qb}") for qb in range(1, QB)]
        wtiles = {}

        def xq_parts(qb, ct):
            # [(moving_ap, lo, hi)] token chunks covering qb's 512 tokens
            if qb == 0:
                return [(xq0_t[0][:, ct, :], 0, 256),
                        (xq0_t[1][:, ct, :], 256, 512)]
            return [(xq_t[qb][:, ct, :], 0, 512)]

        def xq_slice(qb, ct, ks):
            # [128, 128] token slice (for the v projection stationary)
            if qb == 0:
                h, off = divmod(ks, 256)
                return xq0_t[h][:, ct, off:off + 128]
            return xq_t[qb][:, ct, ks:ks + 128]

        # wk0 + xq0a gate the first projection: first on their queues
        nc.sync.dma_start(out=xq0_t[0], in_=xq0a_d[:, :, :])
        wk_ts, wq_ts = [], []
        for ht in range(HDT):
            for name, dram, lst in (("wk", wk_d, wk_ts), ("wq", wq_d,
                                                          wq_ts)):
                t = w_pool.tile([128, CT, 128], BF16, name=f"{name}{ht}",
                                tag=f"{name}{ht}")
                nc.scalar.dma_start(out=t, in_=dram[ht])
                lst.append(t)
        wtiles["wk"], wtiles["wq"] = wk_ts, wq_ts
        nc.sync.dma_start(out=xq0_t[1], in_=xq0b_d[:, :, :])
        for qb in (1, 2):
            nc.sync.dma_start(out=xq_t[qb], in_=xq_d[qb])
        wv_t = w_pool.tile([128, CT, HD], BF16, name="wv", tag="wv")
        nc.scalar.dma_start(out=wv_t, in_=wv_d[:, :, :])
        wtiles["wv"] = wv_t
        nc.scalar.dma_start(out=(xq_t[3]), in_=xq_d[3])
        wp_full = w_pool.tile([128, HDT, C], BF16, name="wp", tag="wp")
        nc.sync.dma_start(out=wp_full, in_=wp_d[:, :, :])

        # preload the exp table set while DMAs land
        warm = const_pool.tile([128, 64], BF16)
        nc.scalar.activation(warm, ones_f, mybir.ActivationFunctionType.Exp,
                             scale=0.0)

        # ---- persistent activations (all bf16) ----
        qT_t = [qk_pool.tile([128, N], BF16, name=f"qT{i}", tag=f"qT{i}")
                for i in range(HDT)]
        kT_t = [qk_pool.tile([128, N], BF16, name=f"kT{i}", tag=f"kT{i}")
                for i in range(HDT)]
        vo_t = [vo_pool.tile([128, HPC * (D + 1)], BF16, name=f"vo{i}",
                             tag=f"vo{i}") for i in range(QT)]
        oht_t = [oht_pool.tile([128, N], BF16, name=f"oht{i}", tag=f"oht{i}")
                 for i in range(HDT)]
        pt_t = [pt_pool.tile([128, 2, 512], BF16, name=f"pt{i}",
                             tag=f"pt{i}") for i in range(QT)]

        for t in vo_t:
            ones_col = t.rearrange("p (h e) -> p h e", h=HPC)[:, :, D:D + 1]
            nc.gpsimd.tensor_copy(
                ones_col, ones_b[:, 0:HPC].rearrange("p (h o) -> p h o", o=1))

        # ---- work quanta ----
        # projections are emitted as ~0.5us pieces so they weave between
        # S-pairs without stalling the S->exp chain (the st double-buffer
        # only tolerates ~2 slots of PE detour).  Pieces of one quantum
        # share a PSUM tile via `holder` and run in deadline order (stable
        # sort keeps same-deadline insertion order).  part-major within a
        # group: a start=True matmul clears has_written for the WHOLE bank,
        # so the two token-half accumulation groups must not interleave
        # (completed data survives the bit-clear).
        def q_projqk_pieces(ht, qb, dst_t, wname):
            w_full = wtiles[wname][ht]
            cs = slice(qb * 512, (qb + 1) * 512)
            holder = {}
            nparts = len(xq_parts(qb, 0))
            step = 2 if nparts == 1 else 4      # ~480ns per piece
            jobs = []          # (pi, ct_lo, ct_hi)
            for pi in range(nparts):
                for clo in range(0, CT, step):
                    jobs.append((pi, clo, clo + step))
            pieces = []
            for ji, (pi, clo, chi) in enumerate(jobs):
                def go(ji=ji, pi=pi, clo=clo, chi=chi):
                    if ji == 0:
                        holder["ps"] = proj_pool.tile(
                            [128, 512], F32, name="proj", tag="proj")
                    ps = holder["ps"]
                    for ct in range(clo, chi):
                        part, lo, hi = xq_parts(qb, ct)[pi]
                        nc.tensor.matmul(
                            ps[:, lo:hi],
                            w_full[:, ct, :],
                            part,
                            start=(ct == 0), stop=(ct == CT - 1))
                    if ji == len(jobs) - 1:
                        nc.vector.tensor_copy(dst_t[ht][:, cs], ps)
                cost = (chi - clo) * PROJ_MM_NS * (256 if nparts == 2
                                                   else 512) / 512.0
                pieces.append((go, cost))
            return pieces

        def push_projqk(dl, ht, qb, dst_t, wname):
            for i, piece in enumerate(q_projqk_pieces(ht, qb, dst_t,
                                                      wname)):
                push(dl + i, (wname, ht, qb, i), piece)

        def q_projv_pieces(kt):
            holder = {}
            pieces = []
            for ji, clo in enumerate(range(0, CT, CT // 2)):
                def go(ji=ji, clo=clo):
                    qbk, off = divmod(kt * 128, 512)
                    if ji == 0:
                        holder["ps"] = proj_pool.tile(
                            [128, 512], F32, name="proj", tag="proj")
                    ps = holder["ps"]
                    for ct in range(clo, clo + CT // 2):
                        nc.tensor.matmul(ps[:, 0:HD],
                                         xq_slice(qbk, ct, off),
                                         wtiles["wv"][:, ct, :],
                                         start=(ct == 0),
                                         stop=(ct == CT - 1))
                    if ji == 1:
                        vo_view = vo_t[kt].rearrange("p (h e) -> p h e",
                                                     h=HPC)
                        ps_view = ps[:, 0:HD].rearrange("p (h d) -> p h d",
                                                        h=HPC)
                        nc.vector.tensor_copy(vo_view[:, :, 0:D], ps_view)
                pieces.append((go, (CT // 2) * PROJ_MM_NS / 2.0))
            return pieces

        def q_projout(qb, ct):
            def go():
                qs = slice(qb * 512, (qb + 1) * 512)
                cs = slice(ct * 128, (ct + 1) * 128)
                ps = proj_pool.tile([128, 512], F32, name="proj", tag="proj")
                for ht in range(HDT):
                    nc.tensor.matmul(
                        ps, wp_full[:, ht, cs], oht_t[ht][:, qs],
                        start=(ht == 0), stop=(ht == HDT - 1))
                yo = out_pool.tile([128, 512], BF16, name="yo", tag="yo")
                if qb >= QB - 2 and ct % 2 == 0:
                    # tail blocks run post-exp: alternate the PSUM->SBUF
                    # casts between the idle ACT engine and DVE
                    nc.scalar.copy(yo, ps)
                else:
                    nc.vector.tensor_copy(yo, ps)
                dma_eng = nc.gpsimd if (qb >= QB - 2 and ct % 2 == 1) \
                    else nc.sync
                dma_eng.dma_start(out=y_d[qb, ct], in_=yo)
            return go, HDT * PROJ_MM_NS

        # ---- slot-stream emission ----
        state = {"pe": 0.0, "act": 0.0}
        fifo = []          # [(deadline, qid, go, cost), ...] kept sorted
        done_ids = set()

        def push(deadline, qid, quantum):
            go, cost = quantum
            fifo.append([deadline, qid, go, cost])
            fifo.sort(key=lambda e: e[0])

        def run_item(item):
            _, qid, go, cost = item
            go()
            state["pe"] += cost
            done_ids.add(qid)

        def force_until(g):
            while fifo and fifo[0][0] <= g:
                run_item(fifo.pop(0))

        def budget_drain():
            while fifo and state["pe"] + fifo[0][3] <= state["act"] + AHEAD_NS:
                run_item(fifo.pop(0))

        # prologue projections for (ht0, qb0): kT tokens 0-255 first (covers
        # kt0/kt1, needs only the xq0a DMA), then the kT remainder, then
        # full qT -- the first S pair only needs the narrow kT chunk + qT
        def projqk_chunk(dst_t, wname, half):
            lo, hi = half * 256, half * 256 + 256
            w_full = wtiles[wname][0]
            ps = proj_pool.tile([128, 512], F32, name="proj", tag="proj")
            for ct in range(CT):
                nc.tensor.matmul(
                    ps[:, 0:256],
                    w_full[:, ct, :],
                    xq0_t[half][:, ct, :],
                    start=(ct == 0), stop=(ct == CT - 1))
            nc.vector.tensor_copy(dst_t[0][:, lo:hi], ps[:, 0:256])
            state["pe"] += CT * PROJ_MM_NS / 2.0

        projqk_chunk(kT_t, "wk", 0)
        projqk_chunk(kT_t, "wk", 1)
        for go, cost in q_projqk_pieces(0, 0, qT_t, "wq"):
            run_item([0, None, go, cost])

        # weave queue: deadlines in global slot units, a few slots before
        # first use; pieces spread one per slot
        for qbk in range(1, QB):
            push_projqk(max(1, 4 * qbk - 4), 0, qbk, kT_t, "wk")
        for kt in range(QT):
            for i, piece in enumerate(q_projv_pieces(kt)):
                push(kt + LAG - 3 + i, ("v", kt, i), piece)
        for qb in range(1, QB):
            push_projqk(16 * qb - 5, 0, qb, qT_t, "wq")
        for qbk in range(QB):
            push_projqk(42 + 5 * qbk, 1, qbk, kT_t, "wk")
        for qb in range(QB):
            push_projqk(64 + 16 * qb - 7, 1, qb, qT_t, "wq")

        iters = [(ht, qb) for ht in range(HDT) for qb in range(QB)]
        ots_by_it = {}
        norm_pending = []
        normb_pending = []
        pv_sched = {}
        for it in range(NIT):
            for kt in range(QT):
                if it == NIT - 1:
                    # drain in 3 waves: early waves run behind already-
                    # completed exps without jamming the in-order PE FIFO
                    # ahead of the last S pairs; the final wave lands after
                    # the last S emission
                    wave = 124 if kt < 4 else (126 if kt < 8 else 128)
                    g = max(it * QT + wave - 112, it * QT + kt + LAST_LAG)
                else:
                    # first PVs wait until the previous iteration's staging
                    # copies (norm_a at +11, ~1.4us on DVE) free the ot
                    # PSUM banks
                    g = it * QT + max(kt + LAG, 14)
                pv_sched.setdefault(g, []).append((it, kt))

        def emit_s(it, kt):
            ht, qb = iters[it]
            qs = slice(qb * 512, (qb + 1) * 512)
            st = st_pool.tile([128, 2, 512], F32, name="st", tag="st",
                              bufs=2)
            # row-tiled halves (tile positions (0,0)/(64,0)) run
            # concurrently on the PE array
            for hp in range(2):
                prow = slice(hp * 64, hp * 64 + 64)
                nc.tensor.matmul(
                    st[:, hp, :],
                    kT_t[ht][prow, kt * 128:(kt + 1) * 128],
                    qT_t[ht][prow, qs])
            state["pe"] += SPAIR_NS
            return st

        def emit_exp(kt, st):
            nc.scalar.activation(
                pt_t[kt], st, mybir.ActivationFunctionType.Exp, scale=SCALE)
            state["act"] += EXP_NS

        def emit_pv(it, kt):
            ht, qb = iters[it]
            if kt == 0:
                ots_by_it[it] = [
                    ot_pool.tile([65, 512], F32, name=f"ot{hp}",
                                 tag=f"ot{hp}", bufs=1)
                    for hp in range(2)]
            ots = ots_by_it[it]
            for hp in range(2):
                h = 2 * ht + hp
                nc.tensor.matmul(
                    ots[hp],
                    vo_t[kt][:, h * (D + 1):(h + 1) * (D + 1)],
                    pt_t[kt][:, hp, :],
                    start=(kt == 0), stop=(kt == QT - 1))
            state["pe"] += 2 * PV_NS
            if kt == QT - 1:
                norm_pending.append(it)

        def emit_norm_a(it):
            # free the PSUM accumulators ASAP on DVE (keeping the ACT
            # engine's exp chain gap-free); one [65,512] copy per head
            # grabs O rows + the ones-column rowsum together
            ots = ots_by_it.pop(it)
            stgs = []
            for hp in range(2):
                stg = small_pool.tile([65, 512], F32, name=f"stg{hp}",
                                      tag=f"stg{hp}")
                if hp == 0 and it == NIT - 1:
                    # post-stream: ACT is idle, run the halves in parallel
                    nc.scalar.copy(stg, ots[hp][0:65, :])
                else:
                    nc.vector.tensor_copy(stg, ots[hp][0:65, :])
                stgs.append(stg)
            return stgs

        def emit_norm_b(it, stgs):
            ht, qb = iters[it]
            qs = slice(qb * 512, (qb + 1) * 512)
            last = it == NIT - 1
            rbs = []
            for hp in range(2):
                # rowsum to its own partition-0 tile: reciprocal_approx_fast
                # breaks on nonzero base partitions as well as on PSUM reads
                sdb = small_pool.tile([1, 512], F32, name=f"sd{hp}",
                                      tag=f"sd{hp}")
                if hp == 0 and last:
                    nc.scalar.copy(sdb, stgs[hp][64:65, :])
                else:
                    nc.vector.tensor_copy(sdb, stgs[hp][64:65, :])
                r32 = small_pool.tile([1, 512], F32, name=f"r32{hp}",
                                      tag=f"r32{hp}")
                nc.vector.reciprocal_approx_fast(r32, sdb)
                # replicate 1/rowsum across partitions on the idle GPSIMD
                # engine (HW-verified bit-faithful); keeps norm off the PE
                rb = small_pool.tile([64, 512], F32, name=f"rb{hp}",
                                     tag=f"rb{hp}")
                nc.gpsimd.partition_broadcast(rb, r32)
                rbs.append(rb)
            for hp in range(2):
                prow = slice(hp * 64, hp * 64 + 64)
                dst = oht_t[ht][prow, qs]
                with nc.allow_low_precision(reason="bf16 attention out"):
                    nc.vector.tensor_mul(dst, stgs[hp][0:64, :], rbs[hp])
            if ht == HDT - 1:
                base = (4 + qb) * 16 + 32
                for ct in range(CT):
                    dl = base + 2 * ct if qb < QB - 1 else 10 ** 6
                    push(dl, ("out", qb, ct), q_projout(qb, ct))

        # slots are emitted in PAIRS: both S-pairs back-to-back (the second
        # pair's half-row weight loads ride the first pair's streams), then
        # both exps, then the lagged PVs -- this halves the costly
        # all-row -> half-row PE transitions
        total_slots = NIT * QT
        for g in range(0, total_slots + 8, 2):
            force_until(g)
            for g2 in (g, g + 1):
                if g2 < total_slots:
                    it, kt = divmod(g2, QT)
                    # exp emitted right after its S pair (ACT queue) while
                    # the two S pairs stay adjacent in the PE FIFO
                    emit_exp(kt, emit_s(it, kt))
            for g2 in (g, g + 1):
                for gpv in pv_sched.pop(g2, ()):
                    emit_pv(*gpv)
            if norm_pending:
                itn = norm_pending.pop(0)
                normb_pending.append((g + 2, itn, emit_norm_a(itn)))
            if normb_pending and normb_pending[0][0] <= g:
                _, itn, sdbs = normb_pending.pop(0)
                emit_norm_b(itn, sdbs)
            budget_drain()
        for g2 in sorted(pv_sched):
            for gpv in pv_sched[g2]:
                emit_pv(*gpv)
        if norm_pending:
            itn = norm_pending.pop(0)
            emit_norm_b(itn, emit_norm_a(itn))
        while normb_pending:
            _, itn, sdbs = normb_pending.pop(0)
            emit_norm_b(itn, sdbs)
        while fifo:
            run_item(fifo.pop(0))

    nc.finalize()
    return nc


_NC_CACHE = None
TRACE = False
LAST_RESULTS = None


def _get_nc():
    global _NC_CACHE
    if _NC_CACHE is None:
        _NC_CACHE = _build()
    return _NC_CACHE


def kernel(x, w_qkv, w_proj, b_proj):
    global LAST_RESULTS
    import ml_dtypes
    from concourse.bass_utils import run_bass_kernel_spmd

    BF = ml_dtypes.bfloat16
    x = np.asarray(x, dtype=np.float32)
    w_qkv = np.asarray(w_qkv, dtype=np.float32)
    w_proj = np.asarray(w_proj, dtype=np.float32)
    b_proj = np.asarray(b_proj, dtype=np.float32)

    nc = _get_nc()

    # xq[qb, p, ct, n] = x[b, qb*512+n, ct*128+p]
    xq_b = []
    for b in range(B):
        xT = x[b].T.astype(BF)                       # [C, N]
        xq_b.append(np.ascontiguousarray(
            xT.reshape(CT, 128, QB, 512).transpose(2, 1, 0, 3)))

    def pre_w(w):                                    # [C, HD] -> [p, ct, HD]
        return np.ascontiguousarray(
            w.reshape(CT, 128, HD).transpose(1, 0, 2).astype(BF))

    def pre_w_ht(w):                           # [C, HD] -> [ht, p, ct, 128]
        return np.ascontiguousarray(
            w.reshape(CT, 128, HDT, 128).transpose(2, 1, 0, 3).astype(BF))

    in_maps = []
    for c in range(NCORES):
        b, g = divmod(c, NCORES // B)
        hs = slice(g * HD, (g + 1) * HD)
        in_maps.append({
            "xq": xq_b[b],
            "xq0a": np.ascontiguousarray(xq_b[b][0][:, :, 0:256]),
            "xq0b": np.ascontiguousarray(xq_b[b][0][:, :, 256:512]),
            "wk": pre_w_ht(w_qkv[:, 1 * C:2 * C][:, hs]),
            "wq": pre_w_ht(w_qkv[:, 0 * C:1 * C][:, hs]),
            "wv": pre_w(w_qkv[:, 2 * C:3 * C][:, hs]),
            # wp[p, ht, c] = w_proj[g*HD + ht*128 + p, c]
            "wp": np.ascontiguousarray(
                w_proj[g * HD:(g + 1) * HD, :]
                .reshape(HDT, 128, C).transpose(1, 0, 2).astype(BF)),
        })
    res = run_bass_kernel_spmd(nc, in_maps, list(range(NCORES)), trace=TRACE)
    LAST_RESULTS = res
    out = np.empty((B, N, C), dtype=np.float32)
    ncb = NCORES // B
    for b in range(B):
        acc = None
        for g in range(ncb):
            y = res.results[b * ncb + g]["y"].astype(np.float32)
            acc = y if acc is None else acc + y
        # y[qb, ct, p, n] -> yT[ct*128+p, qb*512+n]
        yT = acc.transpose(1, 2, 0, 3).reshape(C, N)
        out[b] = yT.T + b_proj
    return out
